# revision 53
# baseline (speedup 1.0000x reference)
"""Trainium2 Bass kernel for DigitConvolutionalModel forward pass.

Model: x[B,784] -> 3x3 valid conv (single channel) -> flatten[676]
       -> relu(.@W1+b1) -> relu(.@W2+b2) -> .@W3+b3 -> [B,10]

Strategy (v4):
  - Pure data parallel: batch 32768 sharded 8 ways (4096 rows/core);
    weights replicated.
  - conv folds into fc1 (host-side 9-tap sparse weight fold, ~0.02% of
    model FLOPs): fc1 contracts K=784 of pixel-major x against
    W1' = C @ W1. All batch compute runs on device in bf16 (fp32 PSUM).
  - Host supplies x pixel-major bf16 ([784, 4096] per core) and reads the
    output back pixel-major ([10, 4096] per core) — zero-FLOP layout
    changes that remove every on-device transpose.
  - fc1's K=16 leftover chunk (784 = 6*128 + 16) is packed: the three
    h-group tail matmuls run concurrently in disjoint 32-row PE groups
    (tile_position), with x[768:784] and W1'[768:784] replicated at
    partition offsets 0/32/64.
  - fc3 keeps hidden-major [10, 512] output (stationary = W3 chunks of
    only 10 columns -> LDWEIGHTS ~free); bias fused in the ScalarE
    eviction; the tile DMAs straight out to the [10, 4096] buffer.
  - Input + weight DMAs split across both HW-DGE rings (SP + Activation)
    so the prologue is not serialized behind one ~200 GB/s queue.
"""

import sys

for _p in (
    "/opt/trn_rl_repo",
    "/root/.axon_site",
    "/root/.axon_site/_ro/trn_rl_repo",
    "/root/.axon_site/_ro/pypackages",
):
    if _p not in sys.path:
        sys.path.append(_p)

from contextlib import ExitStack

import numpy as np
import ml_dtypes

import concourse.bass as bass
import concourse.tile as tile
from concourse import mybir
from concourse.bass_utils import run_bass_kernel_spmd

F32 = mybir.dt.float32
BF16 = mybir.dt.bfloat16
AFT = mybir.ActivationFunctionType

B_FULL = 32768
N_CORES = 8
B_CORE = B_FULL // N_CORES  # 4096
IMG = 28
OHW = 26
FLAT = OHW * OHW  # 676
NPIX = IMG * IMG  # 784
HID = 300
NCLS = 10

BT = 512  # batch tile (matmul moving free dim)
NBT = B_CORE // BT  # 8

NFULL = 6  # full 128-row pixel chunks; chunk 6 is the 16-row leftover
PIX_CH = [(s, min(128, NPIX - s)) for s in range(0, NPIX, 128)]  # 7 chunks
H_CH = [(s, min(128, HID - s)) for s in range(0, HID, 128)]  # 3 chunks


def _legalize_single_wait(nc):
    """This walrus build accepts only one sync-wait per instruction; move
    extra waits onto NoOps inserted just before, on the same engine."""
    n = 0
    for fn in nc.m.functions:
        for bb in fn.blocks:
            new_insts = []
            for inst in bb.instructions:
                si = inst.sync_info
                if si is not None and si.on_wait and len(si.on_wait) > 1:
                    waits = list(si.on_wait)
                    for w in waits[:-1]:
                        nop = mybir.InstNoOp(
                            name=f"{inst.name}-w{n}",
                            sync_info=mybir.SyncInfo(on_wait=[w], on_update=[]),
                            bass_nofuse=True,
                            engine=inst.engine,
                        )
                        n += 1
                        nc.register_instruction(nop, overwrite=True)
                        new_insts.append(nop)
                    inst.sync_info = mybir.SyncInfo(
                        on_wait=[waits[-1]], on_update=list(si.on_update)
                    )
                new_insts.append(inst)
            bb.instructions = new_insts
    return n


def _emit(ctx: ExitStack, tc: tile.TileContext, xt, x6_d, wpk_d, out):
    nc = tc.nc

    const = ctx.enter_context(tc.tile_pool(name="const", bufs=1))
    ps1 = ctx.enter_context(tc.tile_pool(name="ps1", bufs=3, space="PSUM"))
    ps2p = ctx.enter_context(tc.tile_pool(name="ps2p", bufs=1, space="PSUM"))
    ps3p = ctx.enter_context(tc.tile_pool(name="ps3p", bufs=2, space="PSUM"))
    xp = ctx.enter_context(tc.tile_pool(name="xp", bufs=4))
    hp_ = ctx.enter_context(tc.tile_pool(name="hp", bufs=2))
    obp = ctx.enter_context(tc.tile_pool(name="obp", bufs=4))

    # PE warmup operand: zeros (values are irrelevant for the HAM clock
    # gate; matmuls just need to keep the array busy ~3.4us).
    wz = const.tile([128, 128], BF16, name="wz")
    nc.vector.memset(wz[:, :], 0)

    # --- x: demand-paced per-tile loads (a sustained full-rate DMA burst
    # trips the chip into P0: PE 2.4 -> 2.0GHz for the rest of the run).
    # Host packs tile t's six pixel chunks side by side per partition row
    # (6KB contiguous -> 3KB packets -> ~300GB/s per ring vs ~50 at 1KB).
    # Halves ride different rings and land concurrently; tile 0's halves
    # lead both queues. ---
    xsegs = {}

    def load_tile(t):
        xga = xp.tile([128, 3 * BT], BF16, name=f"xa_{t}", tag="xa")
        nc.sync.dma_start(xga[:, :], xt[t * 128 : (t + 1) * 128, 0 : 3 * BT])
        xgb = xp.tile([128, 3 * BT], BF16, name=f"xb_{t}", tag="xb")
        nc.scalar.dma_start(xgb[:, :], xt[t * 128 : (t + 1) * 128, 3 * BT : 6 * BT])
        xsegs[t] = [xga[:, pc * BT : (pc + 1) * BT] for pc in range(3)] + [
            xgb[:, pc * BT : (pc + 1) * BT] for pc in range(3)
        ]

    # tile 0 loads are inlined so each ring's queue order is exactly:
    # sync: wqA, xga_0, xga_1...; scalar: xgb_0, wB, xgb_1... — chunks 3-5
    # land first (scalar ring carries only 0.39MB ahead of them) and tile
    # 0 consumes chunks in arrival order.
    xga0 = xp.tile([128, 3 * BT], BF16, name="xa_0", tag="xa")
    xgb0 = xp.tile([128, 3 * BT], BF16, name="xb_0", tag="xb")
    nc.scalar.dma_start(xgb0[:, :], xt[0:128, 3 * BT : 6 * BT])
    xsegs[0] = [xga0[:, pc * BT : (pc + 1) * BT] for pc in range(3)] + [
        xgb0[:, pc * BT : (pc + 1) * BT] for pc in range(3)
    ]

    # --- weight layout (host-packed, bf16).  Hidden blocks are WIDENED to
    # 364 cols: [h0 0:128 | h1 128:256 | h2 256:300 | zeros | h2-copy
    # 320:364].  The h2-copy makes every h2-producing matmul write its
    # PSUM rows to partitions 64-107 as well, so the 44-row consumers can
    # run row-tiled at PE offset 64 concurrently with an offset-0 matmul
    # — a free replica, no extra slots or DMAs.
    # wpk cols: [0,7) biases (b1 x3, b2 x3, b3; chunk-2 cols also
    # replicated at rows 64-107), [7,2191) six w1p blocks, [2191,2555)
    # w1p6 block (16 rows replicated at partition offsets 0/32/64),
    # [2555,3647) three w2 k-blocks (k2 block rows 64-107 = replica),
    # [3647,3677) three w3 blocks (k2 block likewise replicated).
    # Dependency tracking is tile-granular, so each DMA gets its own SBUF
    # tile; the SP ring (sync) reliably starts first and carries tile 0's
    # first half + the w1p chunks 0-2.
    H_SL = ((0, 128, 128), (128, 256, 128), (256, 364, 108))  # lo, hi, outw
    # One wide-row tile per ring: 5.1KB/2.2KB rows keep packets big (tiny
    # per-chunk tiles measured ~5x slower — ~250ns/packet floor).
    wqA = const.tile([128, 2555], BF16, name="wqA")
    nc.sync.dma_start(wqA[:, :], wpk_d[:, 0:2555])
    nc.sync.dma_start(xga0[:, :], xt[0:128, 0 : 3 * BT])
    wB = const.tile([128, 1122], BF16, name="wB")
    nc.scalar.dma_start(wB[:, :], wpk_d[:, 2555:3677])
    wq = [wqA[:, 7 + pc * 364 : 7 + (pc + 1) * 364] for pc in range(NFULL)]
    wq6 = wqA[0:80, 2191:2555]
    # x6 replicated to partition offsets 0/32/64 so the three fc1 tail
    # matmuls run in disjoint PE row groups concurrently (1 slot, not 3)
    # x6 rides the Activation ring, NOT SWDGE: concurrent SWDGE traffic
    # throttles the SP ring ~5x during the prologue (+4us first matmul).
    x6t = const.tile([80, B_CORE], BF16, name="x6t")
    for r in range(3):
        nc.scalar.dma_start(x6t[32 * r : 32 * r + 16, :], x6_d[:, :])
    # one DVE copy up-converts the bf16 biases to f32 for the bias operands
    bcv = const.tile([128, 7], F32, name="bcv")
    nc.vector.tensor_copy(bcv[:, :], wqA[:, 0:7])
    b1s = [bcv[0 : H_SL[hc][2], hc : hc + 1] for hc in range(3)]
    b2s = [bcv[0 : H_SL[hc][2], 3 + hc : 4 + hc] for hc in range(3)]
    b3s = bcv[0:NCLS, 6:7]

    load_tile(1)
    load_tile(2)

    # warmup burst emitted after the DMA kickoffs so the PE has work while
    # they land: ~4us of junk matmuls release the HAM clock gate so real
    # compute starts at 2.4GHz right as the first x tile arrives.
    warm = ps1.tile([128, 512], F32, name="warm", tag="f1")
    for _ in range(42):
        nc.tensor.matmul(
            warm[0:128, 0:128], wz[:, 0:128], wz[:, 0:128],
            start=True, stop=True,
        )

    MM = nc.tensor.matmul
    W2B = [2 * 364, 2 * 364 + 128, 2 * 364 + 256]  # k2-block g0/g1/g2x col starts

    def evict(eng, h, ps, rows, n, bias):
        if eng is nc.vector:
            nc.vector.tensor_scalar(
                h[0:rows, 0:n], ps[0:rows, 0:n], bias[:, :], 0.0,
                mybir.AluOpType.add, mybir.AluOpType.max,
            )
        else:
            nc.scalar.activation(
                h[0:rows, 0:n], ps[0:rows, 0:n], AFT.Relu, bias=bias[:, :]
            )

    def emit_ob(k2i):
        """Close out a deferred fc3 accumulation: its k2 ran in the pack
        slot just emitted; evict + store."""
        ps3, c0p, offp, np_ = k2i
        ob = obp.tile([NCLS, BT], F32, name="ob", tag="ob")
        nc.scalar.activation(
            ob[:, 0:np_], ps3[0:NCLS, 0:np_], AFT.Identity, bias=b3s[:, :]
        )
        nc.sync.dma_start(out[:, c0p + offp : c0p + offp + np_], ob[:, 0:np_])

    def fc12(t, c0, off, n, k2h2, k2i):
        """fc1+fc2 for batch cols [off,off+n) of tile t; hosts the deferred
        fc3-k2 of two iterations ago (k2h2/k2i) in its second pack slot."""
        xs = xsegs[t]
        cl, cr = c0 + off, c0 + off + n
        # fc1: the three 16-row tail matmuls go first, back-to-back, in
        # disjoint PE row groups (offsets 0/32/64) -> one concurrent slot.
        # Tile 0 instead consumes pixel chunks in arrival order (pc-outer)
        # and takes the tails last, since x6/weights trickle in.
        pss = [ps1.tile([128, 512], F32, name=f"ps1_{hc}", tag="f1") for hc in range(3)]
        def tail(hc):
            lo, hi, ow = H_SL[hc]
            MM(pss[hc][0:ow, 0:n], wq6[32 * hc : 32 * hc + 16, lo:hi],
               x6t[32 * hc : 32 * hc + 16, cl:cr],
               start=(t > 0), stop=(t == 0), skip_group_check=True)
        def full(hc, pc, start, stop):
            lo, hi, ow = H_SL[hc]
            MM(pss[hc][0:ow, 0:n], wq[pc][0:128, lo:hi], xs[pc][0:128, off:off + n],
               start=start, stop=stop, skip_group_check=True)
        if t == 0:
            # pc-outer in DMA-arrival order (scalar ring's chunks 3-5
            # land ~1.3us before sync's 0-2); tails last (x6 on SWDGE)
            for j, pc in enumerate((3, 4, 5, 0, 1, 2)):
                for hc in range(3):
                    full(hc, pc, j == 0, False)
            for hc in range(3):
                tail(hc)
        else:
            for hc in range(3):
                tail(hc)
            for hc in range(3):
                for pc in range(NFULL):
                    full(hc, pc, False, pc == NFULL - 1)
        h1 = []
        for hc in range(3):
            ow = H_SL[hc][2]
            h = hp_.tile([ow, BT], BF16, name=f"h1_{hc}", tag=f"h1_{hc}")
            evict(nc.vector if hc == 0 else nc.scalar, h, pss[hc], ow, n, b1s[hc])
            h1.append(h)

        # fc2: six full matmuls (m-outer, bank-contiguous), then the three
        # 44-row k2 matmuls packed into two slots via PE row tiling; the
        # second slot also hosts the deferred fc3-k2 at row offset 64.
        ps2 = [
            ps2p.tile([128, 512], F32, name=f"ps2_{g}", tag=f"g{g}")
            for g in range(3)
        ]
        for g in range(3):
            lo, hi, ow = H_SL[g]
            for k in (0, 1):
                MM(ps2[g][0:ow, 0:n], wB[0:128, k * 364 + lo : k * 364 + hi],
                   h1[k][0:128, 0:n], start=(k == 0), stop=False,
                   skip_group_check=True)
        MM(ps2[0][0:128, 0:n], wB[0:44, W2B[0] : W2B[0] + 128], h1[2][0:44, 0:n],
           start=False, stop=True, skip_group_check=True)
        MM(ps2[1][0:128, 0:n], wB[64:108, W2B[1] : W2B[1] + 128], h1[2][64:108, 0:n],
           start=False, stop=True, skip_group_check=True)
        MM(ps2[2][0:108, 0:n], wB[0:44, W2B[2] : W2B[2] + 108], h1[2][0:44, 0:n],
           start=False, stop=True, skip_group_check=True)
        if k2i is not None:
            np_ = k2i[3]
            MM(k2i[0][0:NCLS, 0:np_], wB[64:108, 1092 + 2 * NCLS : 1092 + 3 * NCLS],
               k2h2[2][64:108, 0:np_], start=False, stop=True, skip_group_check=True)
            emit_ob(k2i)
        h2 = []
        for g in range(3):
            ow = H_SL[g][2]
            h = hp_.tile([ow, BT], BF16, name=f"h2_{g}", tag=f"h2_{g}")
            evict(nc.vector, h, ps2[g], ow, n, b2s[g])
            h2.append(h)
        return h2

    def fc3part(h2, c0, off, n):
        """fc3 k0+k1 (full 128-row chunks); k2 joins a later pack slot."""
        ps = ps3p.tile([NCLS, 512], F32, name="ps3", tag="f3")
        for k in (0, 1):
            MM(ps[0:NCLS, 0:n], wB[0:128, 1092 + k * NCLS : 1092 + (k + 1) * NCLS],
               h2[k][0:128, 0:n], start=(k == 0), stop=False, skip_group_check=True)
        return (ps, c0, off, n)

    def fc3k2_standalone(h2p, k2i):
        ps3, _, _, np_ = k2i
        MM(ps3[0:NCLS, 0:np_], wB[0:44, 1092 + 2 * NCLS : 1092 + 3 * NCLS],
           h2p[2][0:44, 0:np_], start=False, stop=True, skip_group_check=True)
        emit_ob(k2i)

    prev = None   # (h2, c0, off, n): fc2 done, fc3 k0/k1 not yet issued
    pend = None   # (h2, k2info): fc3 k0/k1 issued, k2 outstanding
    for t in range(NBT):
        c0 = t * BT
        if t + 3 < NBT:
            load_tile(t + 3)
        halves = ((0, 256), (256, 256)) if t == NBT - 1 else ((0, BT),)
        for off, n in halves:
            h2 = fc12(t, c0, off, n, pend[0] if pend else None,
                      pend[1] if pend else None)
            pend = None
            if prev is not None:
                k2i = fc3part(*prev)
                pend = (prev[0], k2i)
            prev = (h2, c0, off, n)
        xsegs.pop(t)
    if pend is not None:
        fc3k2_standalone(pend[0], pend[1])
    k2i = fc3part(*prev)
    fc3k2_standalone(prev[0], k2i)


def _fold_w1(conv_w: np.ndarray, W1: np.ndarray) -> np.ndarray:
    """W1' = C @ W1 via the 9-tap sparse form: 9 scaled slice-adds."""
    W1m = W1.reshape(OHW, OHW, HID)
    out = np.zeros((IMG, IMG, HID), np.float32)
    for dy in range(3):
        for dx in range(3):
            out[dy : dy + OHW, dx : dx + OHW, :] += conv_w[dy, dx] * W1m
    return out.reshape(NPIX, HID)


_NC_CACHE: list = []


def _get_nc():
    if _NC_CACHE:
        return _NC_CACHE[0]
    nc = bass.Bass("TRN2", target_bir_lowering=False, debug=False)
    # xt rows t*128+p hold tile t's six full pixel chunks side by side:
    # [x[pc*128+p, t*512:(t+1)*512] for pc in 0..5] = 6KB contiguous.
    xt = nc.dram_tensor("xt", [NBT * 128, 6 * BT], BF16, kind="ExternalInput").ap()
    x6 = nc.dram_tensor("x6", [16, B_CORE], BF16, kind="ExternalInput").ap()
    wpk = nc.dram_tensor("wpk", [128, 3677], BF16, kind="ExternalInput").ap()
    out = nc.dram_tensor("out", [NCLS, B_CORE], F32, kind="ExternalOutput").ap()
    with tile.TileContext(nc) as tc:
        with ExitStack() as ctx:
            _emit(ctx, tc, xt, x6, wpk, out)
    _legalize_single_wait(nc)
    _NC_CACHE.append(nc)
    return nc


def _in_maps(inputs: dict) -> list:
    x = np.asarray(inputs["x"], dtype=np.float32)
    assert x.shape == (B_FULL, NPIX), x.shape
    bf = ml_dtypes.bfloat16
    # pixel-major per-core layout: [8, 784, 4096] bf16 (zero-FLOP reshape)
    xtp = x.reshape(N_CORES, B_CORE, NPIX).transpose(0, 2, 1).astype(bf)
    # big-packet tile layout: [core][t*128+p, pc*512+j] = xtp[core, pc*128+p,
    # t*512+j] -> every DMA packet is a 6KB contiguous row
    xta = np.ascontiguousarray(
        xtp[:, :768, :]
        .reshape(N_CORES, NFULL, 128, NBT, BT)
        .transpose(0, 3, 2, 1, 4)
        .reshape(N_CORES, NBT * 128, NFULL * BT)
    )
    x6t = np.ascontiguousarray(xtp[:, 768:784, :])
    w1f = _fold_w1(
        np.asarray(inputs["conv_w"], np.float32),
        np.asarray(inputs["W1"], np.float32),
    ).astype(bf)
    W2 = np.asarray(inputs["W2"], np.float32)
    W3 = np.asarray(inputs["W3"], np.float32)
    # packed weight tile: biases | six widened w1p blocks | w1p6 block |
    # three widened w2 k-blocks | three w3 blocks.  "Widened" blocks put a
    # copy of the h2 (44-wide) columns at local cols 320:364 so the h2
    # PSUM rows also materialize at partitions 64-107 (see _emit).
    def widen(M):
        o = np.zeros((M.shape[0], 364), np.float32)
        o[:, 0:256] = M[:, 0:256]
        o[:, 256:300] = M[:, 256:300]
        o[:, 320:364] = M[:, 256:300]
        return o.astype(bf)

    wpk = np.zeros((128, 3677), bf)
    b1 = np.asarray(inputs["b1"], np.float32)
    b2 = np.asarray(inputs["b2"], np.float32)
    for hc, (h0, hp) in enumerate(H_CH):
        wpk[0:hp, hc] = b1[h0 : h0 + hp].astype(bf)
        wpk[0:hp, 3 + hc] = b2[h0 : h0 + hp].astype(bf)
    wpk[64:108, 2] = b1[256:300].astype(bf)
    wpk[64:108, 5] = b2[256:300].astype(bf)
    wpk[0:NCLS, 6] = np.asarray(inputs["b3"], np.float32).astype(bf)
    for pc in range(NFULL):
        wpk[:, 7 + pc * 364 : 7 + (pc + 1) * 364] = widen(w1f[pc * 128 : (pc + 1) * 128])
    w6w = widen(w1f[768:784])
    for r in range(3):
        wpk[32 * r : 32 * r + 16, 2191:2555] = w6w
    for k, (k0, kp) in enumerate(H_CH):
        wpk[0:kp, 2555 + k * 364 : 2555 + (k + 1) * 364] = widen(W2[k0 : k0 + kp])
        wpk[0:kp, 3647 + k * NCLS : 3647 + (k + 1) * NCLS] = W3[k0 : k0 + kp].astype(bf)
    wpk[64:108, 2555 + 2 * 364 : 2555 + 3 * 364] = widen(W2[256:300])
    wpk[64:108, 3647 + 2 * NCLS : 3647 + 3 * NCLS] = W3[256:300].astype(bf)
    common = {"wpk": wpk}
    return [{"xt": xta[c], "x6": x6t[c], **common} for c in range(N_CORES)]


def kernel(**inputs) -> np.ndarray:
    nc = _get_nc()
    res = run_bass_kernel_spmd(nc, _in_maps(inputs), list(range(N_CORES)))
    return np.concatenate(
        [res.results[c]["out"].T for c in range(N_CORES)], axis=0
    )


if __name__ == "__main__":
    rng = np.random.default_rng(0)
    ins = {
        "x": rng.standard_normal((B_FULL, NPIX), dtype=np.float32),
        "conv_w": rng.standard_normal((3, 3), dtype=np.float32) * 0.1,
        "W1": rng.standard_normal((FLAT, HID), dtype=np.float32) * 0.04,
        "b1": np.zeros(HID, np.float32),
        "W2": rng.standard_normal((HID, HID), dtype=np.float32) * 0.06,
        "b2": np.zeros(HID, np.float32),
        "W3": rng.standard_normal((HID, NCLS), dtype=np.float32) * 0.06,
        "b3": np.zeros(NCLS, np.float32),
    }
    y = kernel(**ins)
    # numpy reference with explicit conv
    from numpy.lib.stride_tricks import sliding_window_view

    img = ins["x"].reshape(-1, IMG, IMG)
    win = sliding_window_view(img, (3, 3), axis=(1, 2))
    conv = np.einsum("bijkl,kl->bij", win, ins["conv_w"]).reshape(-1, FLAT)
    h = np.maximum(conv @ ins["W1"] + ins["b1"], 0)
    h = np.maximum(h @ ins["W2"] + ins["b2"], 0)
    ref = h @ ins["W3"] + ins["b3"]
    err = np.abs(y - ref).max() / (np.abs(ref).max() + 1e-9)
    print("max rel err vs numpy:", err)

